# revision 26
# baseline (speedup 1.0000x reference)
"""WaveNet-style gated residual block (AdvancedSkipResidualBlock) on 8 TRN2 NeuronCores.

Strategy: data-parallel over batch B=8 -> one batch element per NeuronCore.
Per core, the whole block is 7 GEMMs of [512,512] weights x [512, T] activations:
  xc   = x + Wc @ cond + bc                      (1 GEMM,  K=512)
  f    = tanh(W_f0 @ xc(t-2) + W_f1 @ xc(t) + bf)  (2 GEMMs, dilated causal conv K=2, d=2)
  g    = sigmoid(W_g0 @ xc(t-2) + W_g1 @ xc(t) + bg)
  h    = f * g
  res  = Wr @ h + br + xc                        (1 GEMM)
  skip = Ws @ h + bs                             (1 GEMM)
Matmuls run in bf16 (fp32 PSUM accumulation); elementwise/bias/activation fused
onto ScalarE (tanh/sigmoid/identity, one LUT table set) and VectorE
(scalar_tensor_tensor fuses bias-add + residual-add in one op).
Time axis is processed in chunks (128/128/256 ramp-up, then 512s) with a
2-column halo for the dilated conv; a 3-stage software pipeline
(cond -> conv/gate -> res/skip) keeps the tensor engine at its issue rate.
Measured: ~223us NEFF exec at full clock (bf16 matmul roofline ~191us/core).
"""

import os
import sys
from contextlib import ExitStack

import numpy as np

try:
    import concourse.bass as bass  # noqa: F401
except ImportError:  # pragma: no cover
    sys.path.insert(0, "/opt/trn_rl_repo")
    import concourse.bass as bass  # noqa: F401

import ml_dtypes  # noqa: E402
import concourse.tile as tile  # noqa: E402
from concourse import bacc, mybir  # noqa: E402
from concourse.bass_utils import run_bass_kernel_spmd  # noqa: E402

B, C, T = 8, 512, 4096
P, G = 128, 4          # SBUF partitions, channel groups (C = G*P)
CH = 512               # steady-state time-chunk width
# Small leading chunks let the PE start while weights/x are still streaming
# in; small trailing chunks shorten the post-matmul drain. Steady-state
# chunks are 512 (one PSUM bank of fp32).
CWS = [128, 128, 256] + [512] * 7
assert sum(CWS) == T
LOS = [0]
for _w in CWS:
    LOS.append(LOS[-1] + _w)
NCH = len(CWS)
DIL = 2                # conv dilation (kernel_size=2 -> taps at t-2 and t)

BF16 = mybir.dt.bfloat16
F32 = mybir.dt.float32
AF = mybir.ActivationFunctionType
ALU = mybir.AluOpType

_CACHE: dict = {}


def _build():
    nc = bacc.Bacc("TRN2", target_bir_lowering=False, debug=False, num_devices=B)

    def din(name, shape, dt):
        return nc.dram_tensor(name, shape, dt, kind="ExternalInput").ap()

    def dout(name, shape, dt):
        return nc.dram_tensor(name, shape, dt, kind="ExternalOutput").ap()

    x_d = din("x", [C, T], BF16)
    c_d = din("cond", [C, T], BF16)
    wc_d = din("wc", [P, G, G, P], BF16)
    wf_d = din("wf", [P, 2, G, G, P], BF16)
    wg_d = din("wg", [P, 2, G, G, P], BF16)
    wr_d = din("wr", [P, G, G, P], BF16)
    ws_d = din("ws", [P, G, G, P], BF16)
    bc_d = din("bc", [P, G], F32)
    bf_d = din("bf", [P, G], F32)
    bg_d = din("bg", [P, G], F32)
    br_d = din("br", [P, G], F32)
    bs_d = din("bs", [P, G], F32)
    r_d = dout("res", [C, T], F32)
    s_d = dout("skip", [C, T], F32)

    x_r = x_d.rearrange("(g p) t -> p g t", p=P)
    c_r = c_d.rearrange("(g p) t -> p g t", p=P)
    r_r = r_d.rearrange("(g p) t -> p g t", p=P)
    s_r = s_d.rearrange("(g p) t -> p g t", p=P)

    with tile.TileContext(nc) as tc, ExitStack() as ctx:
        const = ctx.enter_context(tc.tile_pool(name="const", bufs=1))
        xin = ctx.enter_context(tc.tile_pool(name="xin", bufs=4))
        cin = ctx.enter_context(tc.tile_pool(name="cin", bufs=4))
        xcp = ctx.enter_context(tc.tile_pool(name="xcp", bufs=4))
        fgp = ctx.enter_context(tc.tile_pool(name="fgp", bufs=4))
        hp = ctx.enter_context(tc.tile_pool(name="hp", bufs=3))
        rop = ctx.enter_context(tc.tile_pool(name="rop", bufs=6))
        sop = ctx.enter_context(tc.tile_pool(name="sop", bufs=6))
        psum = ctx.enter_context(tc.tile_pool(name="psum", bufs=8, space="PSUM"))

        # Startup choreography. The first PE work (cond matmuls of chunk 0)
        # needs cond(0) + wc; the fg matmuls soon after need x0 then wf/wg.
        # Sync (HWDGE) queue: c0, wc, x0, biases in that order; the big
        # conv weights stream concurrently on the GpSimd (SWDGE) queue so
        # neither queue serializes the other's critical bytes. Output
        # stores ride the Scalar (HWDGE) queue.
        from concourse.tile_rust import add_dep_helper

        ct0 = cin.tile([P, G, CWS[0]], BF16, tag="c")
        nc.sync.dma_start(ct0[:], c_r[:, :, 0:CWS[0]])
        wc_sb = const.tile([P, G, G, P], BF16)
        i_wc = nc.sync.dma_start(wc_sb[:], wc_d)
        xt0 = xin.tile([P, G, CWS[0]], BF16, tag="x")
        nc.gpsimd.dma_start(xt0[:], x_r[:, :, 0:CWS[0]])
        # Conv weights, split per tap so the first f-matmuls only wait on
        # tap 0. The first conv-weight DMA is held behind wc's completion
        # so the SWDGE stream doesn't steal HBM bandwidth from the bytes
        # that gate the very first matmul.
        wf_tap = []
        wg_tap = []
        for tap in range(2):
            t = const.tile([P, G, G, P], BF16, tag=f"wf{tap}")
            i = nc.gpsimd.dma_start(t[:], wf_d[:, tap])
            if tap == 0:
                add_dep_helper(i.ins, i_wc.ins, reason="prioritize wc bytes at startup")
            wf_tap.append(t)
        for tap in range(2):
            t = const.tile([P, G, G, P], BF16, tag=f"wg{tap}")
            nc.gpsimd.dma_start(t[:], wg_d[:, tap])
            wg_tap.append(t)
        wr_sb = const.tile([P, G, G, P], BF16)
        nc.gpsimd.dma_start(wr_sb[:], wr_d)
        ws_sb = const.tile([P, G, G, P], BF16)
        nc.gpsimd.dma_start(ws_sb[:], ws_d)
        pre_c = {0: ct0}
        pre_x = {0: xt0}
        bias_sb = {}
        for name, ap in (("bc", bc_d), ("bf", bf_d), ("bg", bg_d), ("br", br_d), ("bs", bs_d)):
            t = const.tile([P, G], F32, tag=f"b_{name}")
            nc.sync.dma_start(t[:], ap)
            bias_sb[name] = t

        xc_t: dict = {}
        h_t: dict = {}
        for it in range(NCH + 2):
            c0, c1, c2 = it, it - 1, it - 2

            # ---- stage 1: condition injection (chunk c0) ----
            if c0 < NCH:
                w, lo = CWS[c0], LOS[c0]
                if c0 in pre_c:
                    ct, xt = pre_c.pop(c0), pre_x.pop(c0)
                else:
                    ct = cin.tile([P, G, w], BF16, tag="c")
                    nc.sync.dma_start(ct[:], c_r[:, :, lo:lo + w])
                    xt = xin.tile([P, G, w], BF16, tag="x")
                    nc.sync.dma_start(xt[:], x_r[:, :, lo:lo + w])
                xc = xcp.tile([P, G, w + DIL], BF16, tag="xc")
                if c0 == 0:
                    nc.vector.memset(xc[:, :, 0:DIL], 0.0)
                else:
                    pw = CWS[c0 - 1]
                    nc.vector.tensor_copy(xc[:, :, 0:DIL], xc_t[c0 - 1][:, :, pw:pw + DIL])
                for m in range(G):
                    ps = psum.tile([P, w], F32, space="PSUM", tag="ps")
                    for k in range(G):
                        nc.tensor.matmul(ps, wc_sb[:, k, m, :], ct[:, k, :],
                                         start=(k == 0), stop=(k == G - 1))
                    # xc[m] = (Wc@cond + bc) + x, cast to bf16
                    nc.vector.scalar_tensor_tensor(
                        xc[:, m, DIL:w + DIL], ps, bias_sb["bc"][:, m:m + 1],
                        xt[:, m, :], ALU.add, ALU.add)
                xc_t[c0] = xc

            # ---- stage 2: dilated conv + gated activation (chunk c1) ----
            if 0 <= c1 < NCH:
                w = CWS[c1]
                xc = xc_t[c1]
                h = hp.tile([P, G, w], BF16, tag="h")
                for m in range(G):
                    pf = psum.tile([P, w], F32, space="PSUM", tag="ps")
                    for k in range(G):
                        nc.tensor.matmul(pf, wf_tap[0][:, k, m, :], xc[:, k, 0:w],
                                         start=(k == 0), stop=False)
                    for k in range(G):
                        nc.tensor.matmul(pf, wf_tap[1][:, k, m, :], xc[:, k, DIL:w + DIL],
                                         start=False, stop=(k == G - 1))
                    fsb = fgp.tile([P, w], BF16, tag="f")
                    nc.scalar.activation(fsb[:], pf, AF.Tanh, bias=bias_sb["bf"][:, m:m + 1])
                    pg = psum.tile([P, w], F32, space="PSUM", tag="ps")
                    for k in range(G):
                        nc.tensor.matmul(pg, wg_tap[0][:, k, m, :], xc[:, k, 0:w],
                                         start=(k == 0), stop=False)
                    for k in range(G):
                        nc.tensor.matmul(pg, wg_tap[1][:, k, m, :], xc[:, k, DIL:w + DIL],
                                         start=False, stop=(k == G - 1))
                    gsb = fgp.tile([P, w], BF16, tag="g")
                    nc.scalar.activation(gsb[:], pg, AF.Sigmoid, bias=bias_sb["bg"][:, m:m + 1])
                    nc.vector.tensor_mul(h[:, m, :], fsb[:], gsb[:])
                h_t[c1] = h

            # ---- stage 3: residual + skip projections (chunk c2) ----
            if 0 <= c2 < NCH:
                w, lo = CWS[c2], LOS[c2]
                h = h_t.pop(c2)
                xc = xc_t.pop(c2)
                for m in range(G):
                    pr = psum.tile([P, w], F32, space="PSUM", tag="ps")
                    for k in range(G):
                        nc.tensor.matmul(pr, wr_sb[:, k, m, :], h[:, k, :],
                                         start=(k == 0), stop=(k == G - 1))
                    # res[m] = (Wr@h + br) + xc; per-m store so the tail
                    # chunk's output DMA starts before all 4 m-tiles finish
                    rt = rop.tile([P, w], F32, tag="r")
                    nc.vector.scalar_tensor_tensor(
                        rt[:], pr, bias_sb["br"][:, m:m + 1],
                        xc[:, m, DIL:w + DIL], ALU.add, ALU.add)
                    nc.scalar.dma_start(r_r[:, m, lo:lo + w], rt[:])
                    pk = psum.tile([P, w], F32, space="PSUM", tag="ps")
                    for k in range(G):
                        nc.tensor.matmul(pk, ws_sb[:, k, m, :], h[:, k, :],
                                         start=(k == 0), stop=(k == G - 1))
                    st = sop.tile([P, w], F32, tag="s")
                    nc.scalar.activation(st[:], pk, AF.Identity,
                                         bias=bias_sb["bs"][:, m:m + 1])
                    nc.scalar.dma_start(s_r[:, m, lo:lo + w], st[:])

    nc.compile()
    return nc


def _get_nc():
    if "nc" not in _CACHE:
        _CACHE["nc"] = _build()
    return _CACHE["nc"]


def _wT1(w):
    # [Cout, Cin, 1] -> lhsT layout [P(cin%P), G(cin//P), G(cout//P), P(cout%P)]
    return np.ascontiguousarray(
        np.asarray(w)[:, :, 0].T.reshape(G, P, G, P).transpose(1, 0, 2, 3)
        .astype(ml_dtypes.bfloat16))


def _wT2(w):
    # [Cout, Cin, 2] -> [P, tap, G(cin//P), G(cout//P), P]
    taps = [np.asarray(w)[:, :, t].T.reshape(G, P, G, P).transpose(1, 0, 2, 3)
            for t in range(2)]
    return np.ascontiguousarray(np.stack(taps, axis=1).astype(ml_dtypes.bfloat16))


def _bias(b):
    return np.ascontiguousarray(np.asarray(b).reshape(G, P).T.astype(np.float32))


def kernel(x, condition, wf, bf, wg, bg, wr, br, ws, bs, wc, bc):
    nc = _get_nc()
    x_bf = np.asarray(x).astype(ml_dtypes.bfloat16)
    cond_bf = np.asarray(condition).astype(ml_dtypes.bfloat16)
    shared = {
        "wc": _wT1(wc), "wf": _wT2(wf), "wg": _wT2(wg),
        "wr": _wT1(wr), "ws": _wT1(ws),
        "bc": _bias(bc), "bf": _bias(bf), "bg": _bias(bg),
        "br": _bias(br), "bs": _bias(bs),
    }
    in_maps = [
        {"x": np.ascontiguousarray(x_bf[i]), "cond": np.ascontiguousarray(cond_bf[i]),
         **shared}
        for i in range(B)
    ]
    res = run_bass_kernel_spmd(
        nc, in_maps, list(range(B)),
        trace=bool(os.environ.get("CC_KERNEL_TRACE")))
    _CACHE["last_results"] = res
    residual = np.stack([res.results[i]["res"] for i in range(B)])
    skip = np.stack([res.results[i]["skip"] for i in range(B)])
    return residual, skip


# revision 27
# speedup vs baseline: 1.1880x; 1.1880x over previous
"""WaveNet-style gated residual block (AdvancedSkipResidualBlock) on 8 TRN2 NeuronCores.

Strategy: data-parallel over batch B=8 -> one batch element per NeuronCore.
Per core, the whole block is 7 GEMMs of [512,512] weights x [512, T] activations:
  xc   = x + Wc @ cond + bc                      (1 GEMM,  K=512)
  f    = tanh(W_f0 @ xc(t-2) + W_f1 @ xc(t) + bf)  (2 GEMMs, dilated causal conv K=2, d=2)
  g    = sigmoid(W_g0 @ xc(t-2) + W_g1 @ xc(t) + bg)
  h    = f * g
  res  = Wr @ h + br + xc                        (1 GEMM)
  skip = Ws @ h + bs                             (1 GEMM)
Matmuls run in bf16 (fp32 PSUM accumulation); elementwise/bias/activation fused
onto ScalarE (tanh/sigmoid/identity, one LUT table set) and VectorE
(scalar_tensor_tensor fuses bias-add + residual-add in one op).
Time axis is processed in chunks (128/128/256 ramp-up, then 512s) with a
2-column halo for the dilated conv; a 3-stage software pipeline
(cond -> conv/gate -> res/skip) keeps the tensor engine at its issue rate.
Measured: ~223us NEFF exec at full clock (bf16 matmul roofline ~191us/core).
"""

import os
import sys
from contextlib import ExitStack

import numpy as np

try:
    import concourse.bass as bass  # noqa: F401
except ImportError:  # pragma: no cover
    sys.path.insert(0, "/opt/trn_rl_repo")
    import concourse.bass as bass  # noqa: F401

import ml_dtypes  # noqa: E402
import concourse.tile as tile  # noqa: E402
from concourse import bacc, mybir  # noqa: E402
from concourse.bass_utils import run_bass_kernel_spmd  # noqa: E402

B, C, T = 8, 512, 4096
P, G = 128, 4          # SBUF partitions, channel groups (C = G*P)
CH = 512               # steady-state time-chunk width
# Small leading chunks let the PE start while weights/x are still streaming
# in; small trailing chunks shorten the post-matmul drain. Steady-state
# chunks are 512 (one PSUM bank of fp32).
CWS = [128, 128, 256] + [512] * 7
assert sum(CWS) == T
LOS = [0]
for _w in CWS:
    LOS.append(LOS[-1] + _w)
NCH = len(CWS)
DIL = 2                # conv dilation (kernel_size=2 -> taps at t-2 and t)

BF16 = mybir.dt.bfloat16
F32 = mybir.dt.float32
AF = mybir.ActivationFunctionType
ALU = mybir.AluOpType

_CACHE: dict = {}


def _build():
    nc = bacc.Bacc("TRN2", target_bir_lowering=False, debug=False, num_devices=B)

    def din(name, shape, dt):
        return nc.dram_tensor(name, shape, dt, kind="ExternalInput").ap()

    def dout(name, shape, dt):
        return nc.dram_tensor(name, shape, dt, kind="ExternalOutput").ap()

    x_d = din("x", [C, T], BF16)
    c_d = din("cond", [C, T], BF16)
    wc_d = din("wc", [P, G, G, P], BF16)
    wf_d = din("wf", [P, 2, G, G, P], BF16)
    wg_d = din("wg", [P, 2, G, G, P], BF16)
    wr_d = din("wr", [P, G, G, P], BF16)
    ws_d = din("ws", [P, G, G, P], BF16)
    bc_d = din("bc", [P, G], F32)
    bf_d = din("bf", [P, G], F32)
    bg_d = din("bg", [P, G], F32)
    br_d = din("br", [P, G], F32)
    bs_d = din("bs", [P, G], F32)
    r_d = dout("res", [C, T], F32)
    s_d = dout("skip", [C, T], F32)

    x_r = x_d.rearrange("(g p) t -> p g t", p=P)
    c_r = c_d.rearrange("(g p) t -> p g t", p=P)
    r_r = r_d.rearrange("(g p) t -> p g t", p=P)
    s_r = s_d.rearrange("(g p) t -> p g t", p=P)

    with tile.TileContext(nc) as tc, ExitStack() as ctx:
        const = ctx.enter_context(tc.tile_pool(name="const", bufs=1))
        xin = ctx.enter_context(tc.tile_pool(name="xin", bufs=4))
        cin = ctx.enter_context(tc.tile_pool(name="cin", bufs=4))
        xcp = ctx.enter_context(tc.tile_pool(name="xcp", bufs=4))
        fgp = ctx.enter_context(tc.tile_pool(name="fgp", bufs=4))
        hp = ctx.enter_context(tc.tile_pool(name="hp", bufs=3))
        rop = ctx.enter_context(tc.tile_pool(name="rop", bufs=6))
        sop = ctx.enter_context(tc.tile_pool(name="sop", bufs=6))
        psum = ctx.enter_context(tc.tile_pool(name="psum", bufs=8, space="PSUM"))

        # Startup choreography. The first PE work (cond matmuls of chunk 0)
        # needs cond(0) + wc; the fg matmuls soon after need x0 then wf/wg.
        # Sync (HWDGE) queue: c0, wc, x0, biases in that order; the big
        # conv weights stream concurrently on the GpSimd (SWDGE) queue so
        # neither queue serializes the other's critical bytes. Output
        # stores ride the Scalar (HWDGE) queue.
        from concourse.tile_rust import add_dep_helper

        ct0 = cin.tile([P, G, CWS[0]], BF16, tag="c")
        nc.sync.dma_start(ct0[:], c_r[:, :, 0:CWS[0]])
        # wc split per k-group: the first cond matmuls start when the first
        # 128KB piece lands and overlap the later pieces' arrival
        wc_k = []
        for k in range(G):
            t = const.tile([P, G, P], BF16, tag=f"wc{k}", name=f"wc{k}")
            i_wc = nc.sync.dma_start(t[:], wc_d[:, k])
            wc_k.append(t)
        xt0 = xin.tile([P, G, CWS[0]], BF16, tag="x")
        nc.gpsimd.dma_start(xt0[:], x_r[:, :, 0:CWS[0]])
        # Conv weights, split per tap so the first f-matmuls only wait on
        # tap 0. The first conv-weight DMA is held behind wc's completion
        # so the SWDGE stream doesn't steal HBM bandwidth from the bytes
        # that gate the very first matmul.
        wf_tap = []
        wg_tap = []
        for tap in range(2):
            t = const.tile([P, G, G, P], BF16, tag=f"wf{tap}")
            i = nc.gpsimd.dma_start(t[:], wf_d[:, tap])
            if tap == 0:
                add_dep_helper(i.ins, i_wc.ins, reason="prioritize wc bytes at startup")
            wf_tap.append(t)
        for tap in range(2):
            t = const.tile([P, G, G, P], BF16, tag=f"wg{tap}")
            nc.gpsimd.dma_start(t[:], wg_d[:, tap])
            wg_tap.append(t)
        wr_sb = const.tile([P, G, G, P], BF16)
        nc.gpsimd.dma_start(wr_sb[:], wr_d)
        ws_sb = const.tile([P, G, G, P], BF16)
        nc.gpsimd.dma_start(ws_sb[:], ws_d)
        pre_c = {0: ct0}
        pre_x = {0: xt0}
        bias_sb = {}
        for name, ap in (("bc", bc_d), ("bf", bf_d), ("bg", bg_d), ("br", br_d), ("bs", bs_d)):
            t = const.tile([P, G], F32, tag=f"b_{name}")
            nc.sync.dma_start(t[:], ap)
            bias_sb[name] = t

        xc_t: dict = {}
        h_t: dict = {}
        for it in range(NCH + 2):
            c0, c1, c2 = it, it - 1, it - 2

            # ---- stage 1: condition injection (chunk c0) ----
            if c0 < NCH:
                w, lo = CWS[c0], LOS[c0]
                if c0 in pre_c:
                    ct, xt = pre_c.pop(c0), pre_x.pop(c0)
                else:
                    ct = cin.tile([P, G, w], BF16, tag="c")
                    nc.sync.dma_start(ct[:], c_r[:, :, lo:lo + w])
                    xt = xin.tile([P, G, w], BF16, tag="x")
                    nc.sync.dma_start(xt[:], x_r[:, :, lo:lo + w])
                xc = xcp.tile([P, G, w + DIL], BF16, tag="xc")
                if c0 == 0:
                    nc.vector.memset(xc[:, :, 0:DIL], 0.0)
                else:
                    pw = CWS[c0 - 1]
                    nc.vector.tensor_copy(xc[:, :, 0:DIL], xc_t[c0 - 1][:, :, pw:pw + DIL])
                # k-outer: matmuls for piece k run while piece k+1 streams in
                pss = [psum.tile([P, w], F32, space="PSUM", tag="ps",
                                 name=f"ps_c{c0}m{m}") for m in range(G)]
                for k in range(G):
                    for m in range(G):
                        nc.tensor.matmul(pss[m], wc_k[k][:, m, :], ct[:, k, :],
                                         start=(k == 0), stop=(k == G - 1))
                for m in range(G):
                    # xc[m] = (Wc@cond + bc) + x, cast to bf16
                    nc.vector.scalar_tensor_tensor(
                        xc[:, m, DIL:w + DIL], pss[m], bias_sb["bc"][:, m:m + 1],
                        xt[:, m, :], ALU.add, ALU.add)
                xc_t[c0] = xc

            # ---- stage 2: dilated conv + gated activation (chunk c1) ----
            if 0 <= c1 < NCH:
                w = CWS[c1]
                xc = xc_t[c1]
                h = hp.tile([P, G, w], BF16, tag="h")
                for m in range(G):
                    pf = psum.tile([P, w], F32, space="PSUM", tag="ps")
                    for k in range(G):
                        nc.tensor.matmul(pf, wf_tap[0][:, k, m, :], xc[:, k, 0:w],
                                         start=(k == 0), stop=False)
                    for k in range(G):
                        nc.tensor.matmul(pf, wf_tap[1][:, k, m, :], xc[:, k, DIL:w + DIL],
                                         start=False, stop=(k == G - 1))
                    fsb = fgp.tile([P, w], BF16, tag="f")
                    nc.scalar.activation(fsb[:], pf, AF.Tanh, bias=bias_sb["bf"][:, m:m + 1])
                    pg = psum.tile([P, w], F32, space="PSUM", tag="ps")
                    for k in range(G):
                        nc.tensor.matmul(pg, wg_tap[0][:, k, m, :], xc[:, k, 0:w],
                                         start=(k == 0), stop=False)
                    for k in range(G):
                        nc.tensor.matmul(pg, wg_tap[1][:, k, m, :], xc[:, k, DIL:w + DIL],
                                         start=False, stop=(k == G - 1))
                    gsb = fgp.tile([P, w], BF16, tag="g")
                    nc.scalar.activation(gsb[:], pg, AF.Sigmoid, bias=bias_sb["bg"][:, m:m + 1])
                    nc.vector.tensor_mul(h[:, m, :], fsb[:], gsb[:])
                h_t[c1] = h

            # ---- stage 3: residual + skip projections (chunk c2) ----
            if 0 <= c2 < NCH:
                w, lo = CWS[c2], LOS[c2]
                h = h_t.pop(c2)
                xc = xc_t.pop(c2)
                for m in range(G):
                    pr = psum.tile([P, w], F32, space="PSUM", tag="ps")
                    for k in range(G):
                        nc.tensor.matmul(pr, wr_sb[:, k, m, :], h[:, k, :],
                                         start=(k == 0), stop=(k == G - 1))
                    # res[m] = (Wr@h + br) + xc; per-m store so the tail
                    # chunk's output DMA starts before all 4 m-tiles finish
                    rt = rop.tile([P, w], F32, tag="r")
                    nc.vector.scalar_tensor_tensor(
                        rt[:], pr, bias_sb["br"][:, m:m + 1],
                        xc[:, m, DIL:w + DIL], ALU.add, ALU.add)
                    nc.scalar.dma_start(r_r[:, m, lo:lo + w], rt[:])
                    pk = psum.tile([P, w], F32, space="PSUM", tag="ps")
                    for k in range(G):
                        nc.tensor.matmul(pk, ws_sb[:, k, m, :], h[:, k, :],
                                         start=(k == 0), stop=(k == G - 1))
                    st = sop.tile([P, w], F32, tag="s")
                    nc.scalar.activation(st[:], pk, AF.Identity,
                                         bias=bias_sb["bs"][:, m:m + 1])
                    nc.scalar.dma_start(s_r[:, m, lo:lo + w], st[:])

    nc.compile()
    return nc


def _get_nc():
    if "nc" not in _CACHE:
        _CACHE["nc"] = _build()
    return _CACHE["nc"]


def _wT1(w):
    # [Cout, Cin, 1] -> lhsT layout [P(cin%P), G(cin//P), G(cout//P), P(cout%P)]
    return np.ascontiguousarray(
        np.asarray(w)[:, :, 0].T.reshape(G, P, G, P).transpose(1, 0, 2, 3)
        .astype(ml_dtypes.bfloat16))


def _wT2(w):
    # [Cout, Cin, 2] -> [P, tap, G(cin//P), G(cout//P), P]
    taps = [np.asarray(w)[:, :, t].T.reshape(G, P, G, P).transpose(1, 0, 2, 3)
            for t in range(2)]
    return np.ascontiguousarray(np.stack(taps, axis=1).astype(ml_dtypes.bfloat16))


def _bias(b):
    return np.ascontiguousarray(np.asarray(b).reshape(G, P).T.astype(np.float32))


def kernel(x, condition, wf, bf, wg, bg, wr, br, ws, bs, wc, bc):
    nc = _get_nc()
    x_bf = np.asarray(x).astype(ml_dtypes.bfloat16)
    cond_bf = np.asarray(condition).astype(ml_dtypes.bfloat16)
    shared = {
        "wc": _wT1(wc), "wf": _wT2(wf), "wg": _wT2(wg),
        "wr": _wT1(wr), "ws": _wT1(ws),
        "bc": _bias(bc), "bf": _bias(bf), "bg": _bias(bg),
        "br": _bias(br), "bs": _bias(bs),
    }
    in_maps = [
        {"x": np.ascontiguousarray(x_bf[i]), "cond": np.ascontiguousarray(cond_bf[i]),
         **shared}
        for i in range(B)
    ]
    res = run_bass_kernel_spmd(
        nc, in_maps, list(range(B)),
        trace=bool(os.environ.get("CC_KERNEL_TRACE")))
    _CACHE["last_results"] = res
    residual = np.stack([res.results[i]["res"] for i in range(B)])
    skip = np.stack([res.results[i]["skip"] for i in range(B)])
    return residual, skip


# revision 28
# speedup vs baseline: 1.2015x; 1.0114x over previous
"""WaveNet-style gated residual block (AdvancedSkipResidualBlock) on 8 TRN2 NeuronCores.

Strategy: data-parallel over batch B=8 -> one batch element per NeuronCore.
Per core, the whole block is 7 GEMMs of [512,512] weights x [512, T] activations:
  xc   = x + Wc @ cond + bc                      (1 GEMM,  K=512)
  f    = tanh(W_f0 @ xc(t-2) + W_f1 @ xc(t) + bf)  (2 GEMMs, dilated causal conv K=2, d=2)
  g    = sigmoid(W_g0 @ xc(t-2) + W_g1 @ xc(t) + bg)
  h    = f * g
  res  = Wr @ h + br + xc                        (1 GEMM)
  skip = Ws @ h + bs                             (1 GEMM)
Matmuls run in bf16 (fp32 PSUM accumulation); elementwise/bias/activation fused
onto ScalarE (tanh/sigmoid/identity, one LUT table set) and VectorE
(scalar_tensor_tensor fuses bias-add + residual-add in one op).
Time axis is processed in chunks (128/128/256 ramp-up, then 512s) with a
2-column halo for the dilated conv; a 3-stage software pipeline
(cond -> conv/gate -> res/skip) keeps the tensor engine at its issue rate.
Measured: ~223us NEFF exec at full clock (bf16 matmul roofline ~191us/core).
"""

import os
import sys
from contextlib import ExitStack

import numpy as np

try:
    import concourse.bass as bass  # noqa: F401
except ImportError:  # pragma: no cover
    sys.path.insert(0, "/opt/trn_rl_repo")
    import concourse.bass as bass  # noqa: F401

import ml_dtypes  # noqa: E402
import concourse.tile as tile  # noqa: E402
from concourse import bacc, mybir  # noqa: E402
from concourse.bass_utils import run_bass_kernel_spmd  # noqa: E402

B, C, T = 8, 512, 4096
P, G = 128, 4          # SBUF partitions, channel groups (C = G*P)
CH = 512               # steady-state time-chunk width
# Small leading chunks let the PE start while weights/x are still streaming
# in; small trailing chunks shorten the post-matmul drain. Steady-state
# chunks are 512 (one PSUM bank of fp32).
CWS = [128, 128, 256] + [512] * 7
assert sum(CWS) == T
LOS = [0]
for _w in CWS:
    LOS.append(LOS[-1] + _w)
NCH = len(CWS)
DIL = 2                # conv dilation (kernel_size=2 -> taps at t-2 and t)

BF16 = mybir.dt.bfloat16
F32 = mybir.dt.float32
AF = mybir.ActivationFunctionType
ALU = mybir.AluOpType

_CACHE: dict = {}


def _build():
    nc = bacc.Bacc("TRN2", target_bir_lowering=False, debug=False, num_devices=B)

    def din(name, shape, dt):
        return nc.dram_tensor(name, shape, dt, kind="ExternalInput").ap()

    def dout(name, shape, dt):
        return nc.dram_tensor(name, shape, dt, kind="ExternalOutput").ap()

    x_d = din("x", [C, T], BF16)
    c_d = din("cond", [C, T], BF16)
    wc_d = din("wc", [P, G, G, P], BF16)
    wf_d = din("wf", [P, 2, G, G, P], BF16)
    wg_d = din("wg", [P, 2, G, G, P], BF16)
    wr_d = din("wr", [P, G, G, P], BF16)
    ws_d = din("ws", [P, G, G, P], BF16)
    bc_d = din("bc", [P, G], F32)
    bf_d = din("bf", [P, G], F32)
    bg_d = din("bg", [P, G], F32)
    br_d = din("br", [P, G], F32)
    bs_d = din("bs", [P, G], F32)
    r_d = dout("res", [C, T], F32)
    s_d = dout("skip", [C, T], F32)

    x_r = x_d.rearrange("(g p) t -> p g t", p=P)
    c_r = c_d.rearrange("(g p) t -> p g t", p=P)
    r_r = r_d.rearrange("(g p) t -> p g t", p=P)
    s_r = s_d.rearrange("(g p) t -> p g t", p=P)

    with tile.TileContext(nc) as tc, ExitStack() as ctx:
        const = ctx.enter_context(tc.tile_pool(name="const", bufs=1))
        xin = ctx.enter_context(tc.tile_pool(name="xin", bufs=4))
        cin = ctx.enter_context(tc.tile_pool(name="cin", bufs=4))
        xcp = ctx.enter_context(tc.tile_pool(name="xcp", bufs=4))
        fgp = ctx.enter_context(tc.tile_pool(name="fgp", bufs=4))
        hp = ctx.enter_context(tc.tile_pool(name="hp", bufs=3))
        rop = ctx.enter_context(tc.tile_pool(name="rop", bufs=6))
        sop = ctx.enter_context(tc.tile_pool(name="sop", bufs=6))
        psum = ctx.enter_context(tc.tile_pool(name="psum", bufs=8, space="PSUM"))

        # Startup choreography. The first PE work (cond matmuls of chunk 0)
        # needs cond(0) + wc; the fg matmuls soon after need x0 then wf/wg.
        # Sync (HWDGE) queue: c0, wc, x0, biases in that order; the big
        # conv weights stream concurrently on the GpSimd (SWDGE) queue so
        # neither queue serializes the other's critical bytes. Output
        # stores ride the Scalar (HWDGE) queue.
        from concourse.tile_rust import add_dep_helper

        ct0 = cin.tile([P, G, CWS[0]], BF16, tag="c")
        nc.sync.dma_start(ct0[:], c_r[:, :, 0:CWS[0]])
        wc_sb = const.tile([P, G, G, P], BF16)
        i_wc = nc.sync.dma_start(wc_sb[:], wc_d)
        xt0 = xin.tile([P, G, CWS[0]], BF16, tag="x")
        nc.gpsimd.dma_start(xt0[:], x_r[:, :, 0:CWS[0]])
        # Conv weights, split per tap so the first f-matmuls only wait on
        # tap 0. The first conv-weight DMA is held behind wc's completion
        # so the SWDGE stream doesn't steal HBM bandwidth from the bytes
        # that gate the very first matmul.
        wf_tap = []
        wg_tap = []
        for tap in range(2):
            t = const.tile([P, G, G, P], BF16, tag=f"wf{tap}")
            i = nc.gpsimd.dma_start(t[:], wf_d[:, tap])
            if tap == 0:
                add_dep_helper(i.ins, i_wc.ins, reason="prioritize wc bytes at startup")
            wf_tap.append(t)
        for tap in range(2):
            t = const.tile([P, G, G, P], BF16, tag=f"wg{tap}")
            nc.gpsimd.dma_start(t[:], wg_d[:, tap])
            wg_tap.append(t)
        wr_sb = const.tile([P, G, G, P], BF16)
        nc.gpsimd.dma_start(wr_sb[:], wr_d)
        ws_sb = const.tile([P, G, G, P], BF16)
        nc.gpsimd.dma_start(ws_sb[:], ws_d)
        pre_c = {0: ct0}
        pre_x = {0: xt0}
        bias_sb = {}
        for name, ap in (("bc", bc_d), ("bf", bf_d), ("bg", bg_d), ("br", br_d), ("bs", bs_d)):
            t = const.tile([P, G], F32, tag=f"b_{name}")
            nc.sync.dma_start(t[:], ap)
            bias_sb[name] = t

        xc_t: dict = {}
        h_t: dict = {}
        for it in range(NCH + 2):
            c0, c1, c2 = it, it - 1, it - 2

            # ---- stage 1: condition injection (chunk c0) ----
            if c0 < NCH:
                w, lo = CWS[c0], LOS[c0]
                if c0 in pre_c:
                    ct, xt = pre_c.pop(c0), pre_x.pop(c0)
                else:
                    ct = cin.tile([P, G, w], BF16, tag="c")
                    nc.sync.dma_start(ct[:], c_r[:, :, lo:lo + w])
                    xt = xin.tile([P, G, w], BF16, tag="x")
                    nc.sync.dma_start(xt[:], x_r[:, :, lo:lo + w])
                xc = xcp.tile([P, G, w + DIL], BF16, tag="xc")
                if c0 == 0:
                    nc.vector.memset(xc[:, :, 0:DIL], 0.0)
                else:
                    pw = CWS[c0 - 1]
                    nc.vector.tensor_copy(xc[:, :, 0:DIL], xc_t[c0 - 1][:, :, pw:pw + DIL])
                for m in range(G):
                    ps = psum.tile([P, w], F32, space="PSUM", tag="ps")
                    for k in range(G):
                        nc.tensor.matmul(ps, wc_sb[:, k, m, :], ct[:, k, :],
                                         start=(k == 0), stop=(k == G - 1))
                    # xc[m] = (Wc@cond + bc) + x, cast to bf16
                    nc.vector.scalar_tensor_tensor(
                        xc[:, m, DIL:w + DIL], ps, bias_sb["bc"][:, m:m + 1],
                        xt[:, m, :], ALU.add, ALU.add)
                xc_t[c0] = xc

            # ---- stage 2: dilated conv + gated activation (chunk c1) ----
            if 0 <= c1 < NCH:
                w = CWS[c1]
                xc = xc_t[c1]
                h = hp.tile([P, G, w], BF16, tag="h")
                for m in range(G):
                    pf = psum.tile([P, w], F32, space="PSUM", tag="ps")
                    for k in range(G):
                        nc.tensor.matmul(pf, wf_tap[0][:, k, m, :], xc[:, k, 0:w],
                                         start=(k == 0), stop=False)
                    for k in range(G):
                        nc.tensor.matmul(pf, wf_tap[1][:, k, m, :], xc[:, k, DIL:w + DIL],
                                         start=False, stop=(k == G - 1))
                    fsb = fgp.tile([P, w], BF16, tag="f")
                    nc.scalar.activation(fsb[:], pf, AF.Tanh, bias=bias_sb["bf"][:, m:m + 1])
                    pg = psum.tile([P, w], F32, space="PSUM", tag="ps")
                    for k in range(G):
                        nc.tensor.matmul(pg, wg_tap[0][:, k, m, :], xc[:, k, 0:w],
                                         start=(k == 0), stop=False)
                    for k in range(G):
                        nc.tensor.matmul(pg, wg_tap[1][:, k, m, :], xc[:, k, DIL:w + DIL],
                                         start=False, stop=(k == G - 1))
                    gsb = fgp.tile([P, w], BF16, tag="g")
                    nc.scalar.activation(gsb[:], pg, AF.Sigmoid, bias=bias_sb["bg"][:, m:m + 1])
                    nc.vector.tensor_mul(h[:, m, :], fsb[:], gsb[:])
                h_t[c1] = h

            # ---- stage 3: residual + skip projections (chunk c2) ----
            if 0 <= c2 < NCH:
                w, lo = CWS[c2], LOS[c2]
                h = h_t.pop(c2)
                xc = xc_t.pop(c2)
                for m in range(G):
                    pr = psum.tile([P, w], F32, space="PSUM", tag="ps")
                    for k in range(G):
                        nc.tensor.matmul(pr, wr_sb[:, k, m, :], h[:, k, :],
                                         start=(k == 0), stop=(k == G - 1))
                    # res[m] = (Wr@h + br) + xc; per-m store so the tail
                    # chunk's output DMA starts before all 4 m-tiles finish
                    rt = rop.tile([P, w], F32, tag="r")
                    nc.vector.scalar_tensor_tensor(
                        rt[:], pr, bias_sb["br"][:, m:m + 1],
                        xc[:, m, DIL:w + DIL], ALU.add, ALU.add)
                    nc.scalar.dma_start(r_r[:, m, lo:lo + w], rt[:])
                    pk = psum.tile([P, w], F32, space="PSUM", tag="ps")
                    for k in range(G):
                        nc.tensor.matmul(pk, ws_sb[:, k, m, :], h[:, k, :],
                                         start=(k == 0), stop=(k == G - 1))
                    st = sop.tile([P, w], F32, tag="s")
                    nc.scalar.activation(st[:], pk, AF.Identity,
                                         bias=bias_sb["bs"][:, m:m + 1])
                    nc.scalar.dma_start(s_r[:, m, lo:lo + w], st[:])

    nc.compile()
    return nc


def _get_nc():
    if "nc" not in _CACHE:
        _CACHE["nc"] = _build()
    return _CACHE["nc"]


def _wT1(w):
    # [Cout, Cin, 1] -> lhsT layout [P(cin%P), G(cin//P), G(cout//P), P(cout%P)]
    return np.ascontiguousarray(
        np.asarray(w)[:, :, 0].T.reshape(G, P, G, P).transpose(1, 0, 2, 3)
        .astype(ml_dtypes.bfloat16))


def _wT2(w):
    # [Cout, Cin, 2] -> [P, tap, G(cin//P), G(cout//P), P]
    taps = [np.asarray(w)[:, :, t].T.reshape(G, P, G, P).transpose(1, 0, 2, 3)
            for t in range(2)]
    return np.ascontiguousarray(np.stack(taps, axis=1).astype(ml_dtypes.bfloat16))


def _bias(b):
    return np.ascontiguousarray(np.asarray(b).reshape(G, P).T.astype(np.float32))


def kernel(x, condition, wf, bf, wg, bg, wr, br, ws, bs, wc, bc):
    nc = _get_nc()
    x_bf = np.asarray(x).astype(ml_dtypes.bfloat16)
    cond_bf = np.asarray(condition).astype(ml_dtypes.bfloat16)
    shared = {
        "wc": _wT1(wc), "wf": _wT2(wf), "wg": _wT2(wg),
        "wr": _wT1(wr), "ws": _wT1(ws),
        "bc": _bias(bc), "bf": _bias(bf), "bg": _bias(bg),
        "br": _bias(br), "bs": _bias(bs),
    }
    in_maps = [
        {"x": np.ascontiguousarray(x_bf[i]), "cond": np.ascontiguousarray(cond_bf[i]),
         **shared}
        for i in range(B)
    ]
    res = run_bass_kernel_spmd(
        nc, in_maps, list(range(B)),
        trace=bool(os.environ.get("CC_KERNEL_TRACE")))
    _CACHE["last_results"] = res
    residual = np.stack([res.results[i]["res"] for i in range(B)])
    skip = np.stack([res.results[i]["skip"] for i in range(B)])
    return residual, skip
